# revision 1
# baseline (speedup 1.0000x reference)
"""GCN layer (message passing) on 8 Trainium2 NeuronCores.

out = relu(((D^-1/2 A D^-1/2) X) @ W.T) + X

Strategy (dst-sharded graph partitioning):
  - Destination nodes sharded across 8 cores (12500 nodes each); every core
    holds the full feature table (random-access gather source) and computes
    its 12500 output rows; the host concatenates.
  - Host-side prep (index-space only): per-edge weight ns2 = norm[src]*norm[dst]
    (both rsqrt-degree norms folded into the edge weight); edges grouped by
    (dst tile of 128 nodes, src bucket of 25000 nodes, src) so each dst tile's
    sources are gathered with dma_gather (int16 indices => src buckets), with
    ascending addresses per stream for HBM locality.
  - Device, per dst tile: up to 4 dma_gather calls pull all edge source rows
    into X (the dominant memory traffic ~216MB/core). The segment-sum runs on
    the tensor engine as  zT[i,d] += X_c[e,i].T @ S_c[e,d]  where
    S_c[e,d] = (d == local_dst[e]) * ns2[e] is built with one fused
    tensor_scalar (is_equal then mult) against a constant iota row matrix.
    Then y[d,o] = zT.T @ W.T on the PE, ReLU on ACT, residual add on DVE.
  - num_idxs per gather is static per (tile, bucket) = max count over the 8
    cores (SPMD same-program constraint), so padding is only the cross-core
    spread (~5%); pad slots gather row 0 of the bucket and are annihilated by
    local_dst = -1 (one-hot row of zeros). Unwritten tail columns of X are
    killed the same way, but the first X pool slots are memzeroed once since
    0 * garbage-NaN would poison PSUM.
"""

import math

import numpy as np

import concourse.bacc as bacc
import concourse.mybir as mybir
from concourse.bass_utils import run_bass_kernel_spmd
from concourse.tile import TileContext

P = 128
N_CORES = 8
BUCKET_MAX = 25000  # int16 gather indices: bucket the node space


def _prepare(features, W, edge_src, edge_dst, n_cores=N_CORES, bucket_max=BUCKET_MAX):
    """Partition the graph by dst core / dst tile / src bucket."""
    features = np.asarray(features, dtype=np.float32)
    W = np.asarray(W, dtype=np.float32)
    edge_src = np.asarray(edge_src, dtype=np.int32)
    edge_dst = np.asarray(edge_dst, dtype=np.int32)

    n_nodes, d = features.shape
    assert d == P
    assert n_nodes % n_cores == 0
    npc = n_nodes // n_cores
    n_tiles = math.ceil(npc / P)
    rows_last = npc - (n_tiles - 1) * P
    nb = math.ceil(n_nodes / bucket_max)
    B = math.ceil(n_nodes / nb)
    assert B <= 32768

    degs = np.bincount(edge_dst, minlength=n_nodes).astype(np.float32)
    norm = 1.0 / np.sqrt(np.maximum(degs, 1.0), dtype=np.float32)
    ns2 = norm[edge_src] * norm[edge_dst]

    core_of = edge_dst // npc

    # first pass: per-core sorted edge lists and per-(tile,bucket) counts
    per_core = []
    counts_all = np.zeros((n_cores, n_tiles, nb), np.int64)
    for k in range(n_cores):
        sel = np.flatnonzero(core_of == k)
        src_k = edge_src[sel]
        ldst = edge_dst[sel] - k * npc
        tile_of = ldst // P
        bucket = src_k // B
        order = np.lexsort((src_k, bucket, tile_of))
        sel = sel[order]
        gid = tile_of[order] * nb + bucket[order]
        counts = np.bincount(gid, minlength=n_tiles * nb).reshape(n_tiles, nb)
        counts_all[k] = counts
        per_core.append((sel, gid, (ldst[order] % P).astype(np.float32)))

    # static per-(tile,bucket) gather sizes: max across cores
    n_tb = counts_all.max(axis=0)  # [n_tiles, nb]
    ct_tb = (n_tb + P - 1) // P  # chunks per (tile, bucket)
    C_t = ct_tb.sum(axis=1)  # chunks per tile
    icols_tb = (n_tb + 15) // 16  # int16 idx columns per (tile, bucket)
    icols_t = icols_tb.sum(axis=1)

    # column offsets in the packed DRAM arrays
    chunk_off_in_tile = np.cumsum(ct_tb, axis=1) - ct_tb  # [n_tiles, nb]
    icol_off_in_tile = np.cumsum(icols_tb, axis=1) - icols_tb
    ldns_col_off = np.concatenate([[0], np.cumsum(3 * C_t)])[:-1]  # per tile
    icol_off_tile = np.concatenate([[0], np.cumsum(icols_t)])[:-1]
    total_icols = int(icols_t.sum())
    total_ldns = int((3 * C_t).sum())

    layout = dict(
        n_nodes=n_nodes,
        npc=npc,
        n_tiles=n_tiles,
        rows_last=rows_last,
        nb=nb,
        B=B,
        n_tb=n_tb,
        ct_tb=ct_tb,
        C_t=C_t,
        icols_tb=icols_tb,
        chunk_off_in_tile=chunk_off_in_tile,
        icol_off_in_tile=icol_off_in_tile,
        ldns_col_off=ldns_col_off,
        icol_off_tile=icol_off_tile,
        total_icols=total_icols,
        total_ldns=total_ldns,
    )

    in_maps = []
    wt = np.ascontiguousarray(W.T)  # wt[i, o] = W[o, i]
    iotam = np.tile(np.arange(P, dtype=np.float32), (P, 1))
    for k in range(n_cores):
        sel, gid, ld_sorted = per_core[k]
        group_start = np.zeros(n_tiles * nb, np.int64)
        cnts = counts_all[k].reshape(-1)
        group_start[1:] = np.cumsum(cnts)[:-1]
        pos = np.arange(len(sel)) - group_start[gid]
        t_of = gid // nb
        b_of = gid % nb

        # idx array [16, total_icols] then replicated to 128 partitions
        idx16 = np.zeros((16, total_icols), np.int16)
        icol = icol_off_tile[t_of] + icol_off_in_tile[t_of, b_of] + pos // 16
        idx16[pos % 16, icol] = (edge_src[sel] - b_of * B).astype(np.int16)
        idxm = np.tile(idx16, (8, 1))

        # ldns array [128, total_ldns]: per tile [ld columns | ns columns]
        ldns = np.zeros((P, total_ldns), np.float32)
        # default ld = -1 in all ld column regions
        for t in range(n_tiles):
            ldns[:, ldns_col_off[t] : ldns_col_off[t] + C_t[t]] = -1.0
        cit = chunk_off_in_tile[t_of, b_of] + pos // P
        e_idx = pos % P
        ldns[e_idx, ldns_col_off[t_of] + cit] = ld_sorted
        ldns[e_idx, ldns_col_off[t_of] + C_t[t_of] + cit] = ns2[sel]
        ldns[e_idx, ldns_col_off[t_of] + 2 * C_t[t_of] + cit] = -ns2[sel]

        in_maps.append(
            {
                "feats": features,
                "idxm": np.ascontiguousarray(idxm),
                "ldns": np.ascontiguousarray(ldns),
                "wt": wt,
                "iotam": iotam,
                "resid": np.ascontiguousarray(features[k * npc : (k + 1) * npc]),
            }
        )
    return in_maps, layout


def _build_program(layout):
    f32 = mybir.dt.float32
    i16 = mybir.dt.int16
    n_nodes = layout["n_nodes"]
    npc = layout["npc"]
    n_tiles = layout["n_tiles"]
    rows_last = layout["rows_last"]
    nb = layout["nb"]
    B = layout["B"]
    n_tb = layout["n_tb"]
    ct_tb = layout["ct_tb"]
    C_t = layout["C_t"]
    icols_tb = layout["icols_tb"]
    chunk_off_in_tile = layout["chunk_off_in_tile"]
    icol_off_in_tile = layout["icol_off_in_tile"]
    ldns_col_off = layout["ldns_col_off"]
    icol_off_tile = layout["icol_off_tile"]
    Cmax = int(C_t.max())

    nc = bacc.Bacc(num_swdge_queues=4)
    feats = nc.declare_dram_parameter("feats", [n_nodes, P], f32, isOutput=False)
    idxm = nc.declare_dram_parameter(
        "idxm", [P, layout["total_icols"]], i16, isOutput=False
    )
    ldns = nc.declare_dram_parameter(
        "ldns", [P, layout["total_ldns"]], f32, isOutput=False
    )
    wt = nc.declare_dram_parameter("wt", [P, P], f32, isOutput=False)
    iotam = nc.declare_dram_parameter("iotam", [P, P], f32, isOutput=False)
    resid = nc.declare_dram_parameter("resid", [npc, P], f32, isOutput=False)
    out = nc.declare_dram_parameter("out", [npc, P], f32, isOutput=True)

    X_BUFS = 3
    with TileContext(nc) as tc:
        with (
            tc.tile_pool(name="const", bufs=1) as constp,
            tc.tile_pool(name="meta", bufs=3) as metap,
            tc.tile_pool(name="x", bufs=X_BUFS) as xp,
            tc.tile_pool(name="s", bufs=6) as sp,
            tc.tile_pool(name="zps", bufs=2, space="PSUM") as zpsp,
            tc.tile_pool(name="yps", bufs=2, space="PSUM") as ypsp,
            tc.tile_pool(name="post", bufs=3) as postp,
        ):
            wt_sb = constp.tile([P, P], f32)
            nc.sync.dma_start(out=wt_sb[:], in_=wt[:, :])
            iota_f = constp.tile([P, P], f32)
            nc.sync.dma_start(out=iota_f[:], in_=iotam[:, :])

            for t in range(n_tiles):
                Ct = int(C_t[t])
                icols = int(icols_tb[t].sum())
                mt_i = metap.tile([P, max(icols, 1)], i16, tag="mi")
                mt_ln = metap.tile([P, 3 * Ct], f32, tag="mldns")
                ic0 = int(icol_off_tile[t])
                nc.sync.dma_start(out=mt_i[:, :icols], in_=idxm[:, ic0 : ic0 + icols])
                lc0 = int(ldns_col_off[t])
                nc.sync.dma_start(out=mt_ln[:], in_=ldns[:, lc0 : lc0 + 3 * Ct])

                # X[e, c*128:(c+1)*128] = feats[gathered src of (chunk c, slot e)]
                X_full = xp.tile([P, Cmax * P], f32, tag="X")
                X = X_full[:, : Ct * P]
                for b in range(nb):
                    n_idx = int(n_tb[t, b])
                    if n_idx == 0:
                        continue
                    co = int(chunk_off_in_tile[t, b])
                    cb = int(ct_tb[t, b])
                    io = int(icol_off_in_tile[t, b])
                    icb = int(icols_tb[t, b])
                    if n_idx % P:
                        # the gather leaves partitions >= n_idx%128 of its
                        # last chunk unwritten; pre-zero that chunk so
                        # 0 * NaN can't poison the one-hot matmul (memzero
                        # bitcasts to uint32 - no NaN read path)
                        nc.scalar.memzero(X[:, (co + cb - 1) * P : (co + cb) * P])
                    nc.gpsimd.dma_gather(
                        out_ap=X[:, co * P : (co + cb) * P].rearrange(
                            "p (c e) -> p c e", e=P
                        ),
                        in_ap=feats[b * B : min((b + 1) * B, n_nodes), :],
                        idxs_ap=mt_i[:, io : io + icb],
                        num_idxs=n_idx,
                        num_idxs_reg=n_idx,
                        elem_size=P,
                        # single_packet concatenates the whole stream into one
                        # SDMA packet; the packet limit is 64 descriptors, and
                        # these calls emit ~70-90 per engine
                        single_packet=False,
                        # one SWDGE queue per bucket: queues run on distinct
                        # Q7 core pairs, parallelizing descriptor generation
                        queue_num=b % 4,
                    )

                z_ps = zpsp.tile([P, P], f32)
                for c in range(Ct):
                    S = sp.tile([P, P], f32, tag="S")
                    # split one-hot builds across DVE and ACT (nc.any piled
                    # all of them onto DVE: 2.9ms busy in the profile).
                    # ACT has no tensor_scalar; for integer iota/ld,
                    # relu(ns - ns*(ld-iota)^2) == (iota==ld)*ns exactly.
                    if c % 2 == 0:
                        nc.vector.tensor_scalar(
                            out=S[:],
                            in0=iota_f[:],
                            scalar1=mt_ln[:, c : c + 1],
                            scalar2=mt_ln[:, Ct + c : Ct + c + 1],
                            op0=mybir.AluOpType.is_equal,
                            op1=mybir.AluOpType.mult,
                        )
                    else:
                        t2 = sp.tile([P, P], f32, tag="T2")
                        nc.scalar.activation(
                            out=t2[:],
                            in_=iota_f[:],
                            func=mybir.ActivationFunctionType.Square,
                            bias=mt_ln[:, c : c + 1],
                            scale=-1.0,
                        )
                        nc.scalar.activation(
                            out=S[:],
                            in_=t2[:],
                            func=mybir.ActivationFunctionType.Relu,
                            bias=mt_ln[:, Ct + c : Ct + c + 1],
                            scale=mt_ln[:, 2 * Ct + c : 2 * Ct + c + 1],
                        )
                    # zT[i, d] += X_c[e, i].T @ S[e, d]
                    nc.tensor.matmul(
                        out=z_ps[:],
                        lhsT=X[:, c * P : (c + 1) * P],
                        rhs=S[:],
                        start=(c == 0),
                        stop=(c == Ct - 1),
                    )

                zT_sb = postp.tile([P, P], f32, tag="zT")
                nc.scalar.copy(out=zT_sb[:], in_=z_ps[:])
                y_ps = ypsp.tile([P, P], f32)
                # y[d, o] = zT[i, d].T @ wt[i, o]
                nc.tensor.matmul(
                    out=y_ps[:], lhsT=zT_sb[:], rhs=wt_sb[:], start=True, stop=True
                )

                rows = P if t < n_tiles - 1 else rows_last
                y_sb = postp.tile([P, P], f32, tag="y")
                nc.scalar.activation(
                    out=y_sb[:], in_=y_ps[:], func=mybir.ActivationFunctionType.Relu
                )
                res_sb = postp.tile([P, P], f32, tag="res")
                nc.sync.dma_start(
                    out=res_sb[:rows], in_=resid[t * P : t * P + rows, :]
                )
                o_sb = postp.tile([P, P], f32, tag="o")
                nc.vector.tensor_add(
                    out=o_sb[:rows], in0=y_sb[:rows], in1=res_sb[:rows]
                )
                nc.sync.dma_start(out=out[t * P : t * P + rows, :], in_=o_sb[:rows])
    nc.finalize()
    return nc


def _run(features, W, edge_src, edge_dst, trace=False, **spmd_kwargs):
    in_maps, layout = _prepare(features, W, edge_src, edge_dst)
    nc = _build_program(layout)
    br = run_bass_kernel_spmd(
        nc, in_maps, core_ids=list(range(N_CORES)), trace=trace, **spmd_kwargs
    )
    outs = [r["out"] for r in br.results]
    full = np.concatenate(outs, axis=0).astype(np.float32)
    return full, br


def kernel(features, W, edge_src, edge_dst):
    out, _ = _run(features, W, edge_src, edge_dst, trace=False)
    return out



# revision 4
# speedup vs baseline: 1.9638x; 1.9638x over previous
"""GCN layer (message passing) on 8 Trainium2 NeuronCores.

out = relu(((D^-1/2 A D^-1/2) X) @ W.T) + X

Strategy (dst-sharded graph partitioning, bf16 gather table):
  - Destination nodes sharded across 8 cores (12500 each); every core holds
    the full feature table and computes its 12500 output rows; host concats.
  - Device prologue: cast the f32 feature table to a bf16 DRAM table h with
    the pre-norm D^-1/2 folded in (h[n] = norm[n] * x[n]); partition p casts
    16 consecutive rows per iteration so DMA descriptors stay contiguous.
    The post-norm norm[dst] is folded into the final ReLU's per-partition
    scale, so the one-hot scatter matrices are pure 0/1.
  - Main loop, per dst tile of 128 nodes: up to 4 dma_gather calls (one per
    src bucket of 25088 nodes, int16 indices) pull the edge source rows as
    bf16 into X [128 slots, Ct*128]; idx streams are padded with -1 which
    the gather ucode strips, so each core only moves its actual edge count.
    The one-hot S [128, Ct*128] bf16 is built in ONE wide DVE tensor_tensor
    (iota broadcast along chunks, ld broadcast along the 128 lane dim, both
    via stride-0 APs), then Ct bf16 matmuls accumulate zT[i,d] in PSUM f32,
    y = relu(norm_dst * (zT.T @ W.T)) on ACT, residual add on DVE.
  - Unwritten X slots (cross-core count spread + chunk padding) are
    memzeroed so NaN garbage can't poison the 0-weighted matmul lanes.
"""

import math

import numpy as np

import concourse.bacc as bacc
import concourse.mybir as mybir
from concourse.bass import AP
from concourse.bass_utils import run_bass_kernel_spmd
from concourse.tile import TileContext

P = 128
N_CORES = 8
NB = 4
B = 25088  # bucket size (multiple of 128, int16-indexable)
NPAD = NB * B  # padded node count 100352
CAST_G = 16  # rows per partition per cast iteration
N_NODES = 100000
SINGLE_PACKET = False


def _prepare(features, W, edge_src, edge_dst, n_cores=N_CORES):
    features = np.asarray(features, dtype=np.float32)
    W = np.asarray(W, dtype=np.float32)
    edge_src = np.asarray(edge_src, dtype=np.int32)
    edge_dst = np.asarray(edge_dst, dtype=np.int32)

    n_nodes, d = features.shape
    assert d == P and n_nodes == N_NODES
    npc = n_nodes // n_cores
    n_tiles = math.ceil(npc / P)
    rows_last = npc - (n_tiles - 1) * P

    degs = np.bincount(edge_dst, minlength=n_nodes).astype(np.float32)
    norm = 1.0 / np.sqrt(np.maximum(degs, 1.0), dtype=np.float32)
    norm_pad = np.zeros(NPAD, np.float32)
    norm_pad[:n_nodes] = norm

    featspad = np.zeros((NPAD, P), np.float32)
    featspad[:n_nodes] = features

    # normP[p, j*CAST_G + g] = norm[j*128*CAST_G + p*CAST_G + g]
    n_cast_cols = NPAD // P  # 784
    normP = norm_pad.reshape(n_cast_cols // CAST_G, P, CAST_G)
    normP = np.ascontiguousarray(normP.transpose(1, 0, 2).reshape(P, n_cast_cols))

    core_of = edge_dst // npc

    per_core = []
    counts_all = np.zeros((n_cores, n_tiles, NB), np.int64)
    for k in range(n_cores):
        sel = np.flatnonzero(core_of == k)
        src_k = edge_src[sel]
        ldst = edge_dst[sel] - k * npc
        tile_of = ldst // P
        bucket = src_k // B
        order = np.lexsort((src_k, bucket, tile_of))
        sel = sel[order]
        gid = tile_of[order] * NB + bucket[order]
        counts = np.bincount(gid, minlength=n_tiles * NB).reshape(n_tiles, NB)
        counts_all[k] = counts
        per_core.append((sel, gid, (ldst[order] % P).astype(np.float32)))

    n_tb = counts_all.max(axis=0)  # static gather sizes [n_tiles, NB]
    cnt_min = counts_all.min(axis=0)  # memzero span start
    assert n_tb.sum(axis=1).min() > 0
    ct_tb = (n_tb + P - 1) // P
    C_t = ct_tb.sum(axis=1)
    icols_tb = (n_tb + 15) // 16
    icols_t = icols_tb.sum(axis=1)

    chunk_off_in_tile = np.cumsum(ct_tb, axis=1) - ct_tb
    icol_off_in_tile = np.cumsum(icols_tb, axis=1) - icols_tb
    ld_col_off = np.concatenate([[0], np.cumsum(C_t)])[:-1]
    icol_off_tile = np.concatenate([[0], np.cumsum(icols_t)])[:-1]
    total_icols = int(icols_t.sum())
    total_C = int(C_t.sum())

    layout = dict(
        npc=npc,
        n_tiles=n_tiles,
        rows_last=rows_last,
        n_tb=n_tb,
        cnt_min=cnt_min,
        ct_tb=ct_tb,
        C_t=C_t,
        icols_tb=icols_tb,
        chunk_off_in_tile=chunk_off_in_tile,
        icol_off_in_tile=icol_off_in_tile,
        ld_col_off=ld_col_off,
        icol_off_tile=icol_off_tile,
        total_icols=total_icols,
        total_C=total_C,
        n_cast_cols=n_cast_cols,
    )

    in_maps = []
    wt = np.ascontiguousarray(W.T)
    iotam = np.tile(np.arange(P, dtype=np.float32), (P, 1))
    for k in range(n_cores):
        sel, gid, ld_sorted = per_core[k]
        group_start = np.zeros(n_tiles * NB, np.int64)
        cnts = counts_all[k].reshape(-1)
        group_start[1:] = np.cumsum(cnts)[:-1]
        pos = np.arange(len(sel)) - group_start[gid]
        t_of = gid // NB
        b_of = gid % NB

        # pad with 0 (gathers bucket row 0; killed by ld=-1 in S). -1 padding
        # (ucode strips trailing negatives) hard-crashes the device at scale.
        idx16 = np.zeros((16, total_icols), np.int16)
        icol = icol_off_tile[t_of] + icol_off_in_tile[t_of, b_of] + pos // 16
        idx16[pos % 16, icol] = (edge_src[sel] - b_of * B).astype(np.int16)
        idxm = np.tile(idx16, (8, 1))

        # ld array [128, total_C]: local dst per (chunk col, slot partition)
        ldm = np.full((P, total_C), -1.0, np.float32)
        cit = chunk_off_in_tile[t_of, b_of] + pos // P
        ldm[pos % P, ld_col_off[t_of] + cit] = ld_sorted

        # normcol[p, t] = norm[k*npc + t*128 + p] (own dst rows)
        nslice = np.zeros(n_tiles * P, np.float32)
        nslice[:npc] = norm[k * npc : (k + 1) * npc]
        normcol = np.ascontiguousarray(nslice.reshape(n_tiles, P).T)

        in_maps.append(
            {
                "featspad": featspad,
                "idxm": np.ascontiguousarray(idxm),
                "ldm": np.ascontiguousarray(ldm),
                "wt": wt,
                "iotam": iotam,
                "normP": normP,
                "normcol": normcol,
                "resid": np.ascontiguousarray(features[k * npc : (k + 1) * npc]),
            }
        )
    return in_maps, layout


def _build_program(layout):
    f32 = mybir.dt.float32
    bf16 = mybir.dt.bfloat16
    i16 = mybir.dt.int16
    npc = layout["npc"]
    n_tiles = layout["n_tiles"]
    rows_last = layout["rows_last"]
    n_tb = layout["n_tb"]
    cnt_min = layout["cnt_min"]
    ct_tb = layout["ct_tb"]
    C_t = layout["C_t"]
    icols_tb = layout["icols_tb"]
    chunk_off_in_tile = layout["chunk_off_in_tile"]
    icol_off_in_tile = layout["icol_off_in_tile"]
    ld_col_off = layout["ld_col_off"]
    icol_off_tile = layout["icol_off_tile"]
    n_cast_cols = layout["n_cast_cols"]
    Cmax = int(C_t.max())

    nc = bacc.Bacc(num_swdge_queues=4)
    featspad = nc.declare_dram_parameter("featspad", [NPAD, P], f32, isOutput=False)
    idxm = nc.declare_dram_parameter(
        "idxm", [P, layout["total_icols"]], i16, isOutput=False
    )
    ldm = nc.declare_dram_parameter("ldm", [P, layout["total_C"]], f32, isOutput=False)
    wt = nc.declare_dram_parameter("wt", [P, P], f32, isOutput=False)
    iotam = nc.declare_dram_parameter("iotam", [P, P], f32, isOutput=False)
    normP = nc.declare_dram_parameter("normP", [P, n_cast_cols], f32, isOutput=False)
    normcol = nc.declare_dram_parameter("normcol", [P, n_tiles], f32, isOutput=False)
    resid = nc.declare_dram_parameter("resid", [npc, P], f32, isOutput=False)
    out = nc.declare_dram_parameter("out", [npc, P], f32, isOutput=True)

    with TileContext(nc) as tc:
        with (
            tc.tile_pool(name="const", bufs=1) as constp,
            tc.tile_pool(name="hdram", bufs=1, space="DRAM") as hp,
            tc.tile_pool(name="cast", bufs=2) as castp,
            tc.tile_pool(name="meta", bufs=3) as metap,
            tc.tile_pool(name="x", bufs=4) as xp,
            tc.tile_pool(name="s", bufs=3) as sp,
            tc.tile_pool(name="zps", bufs=2, space="PSUM") as zpsp,
            tc.tile_pool(name="yps", bufs=2, space="PSUM") as ypsp,
            tc.tile_pool(name="post", bufs=3) as postp,
        ):
            wt_f = constp.tile([P, P], f32)
            nc.sync.dma_start(out=wt_f[:], in_=wt[:, :])
            wt_bf = constp.tile([P, P], bf16)
            nc.scalar.copy(out=wt_bf[:], in_=wt_f[:])
            iota_f = constp.tile([P, P], f32)
            nc.sync.dma_start(out=iota_f[:], in_=iotam[:, :])
            iota_bf = constp.tile([P, P], bf16)
            nc.scalar.copy(out=iota_bf[:], in_=iota_f[:])
            normP_sb = constp.tile([P, n_cast_cols], f32)
            nc.sync.dma_start(out=normP_sb[:], in_=normP[:, :])
            normcol_sb = constp.tile([P, n_tiles], f32)
            nc.sync.dma_start(out=normcol_sb[:], in_=normcol[:, :])

            hfull = hp.tile([NPAD, P], bf16)

            # ---- prologue: h = norm * x, f32 -> bf16, 2048 rows/iter ----
            for j in range(n_cast_cols // CAST_G):
                r0 = j * P * CAST_G
                cin = castp.tile([P, CAST_G * P], f32, tag="cin")
                nc.sync.dma_start(
                    out=cin[:].rearrange("p (g d) -> p g d", d=P),
                    in_=featspad[r0 : r0 + P * CAST_G, :].rearrange(
                        "(p g) d -> p g d", g=CAST_G
                    ),
                )
                cout = castp.tile([P, CAST_G * P], bf16, tag="cout")
                nrm = normP_sb[:, j * CAST_G : (j + 1) * CAST_G]
                nc.vector.scalar_tensor_tensor(
                    out=cout[:].rearrange("p (g d) -> p g d", d=P),
                    in0=cin[:].rearrange("p (g d) -> p g d", d=P),
                    scalar=1.0,
                    in1=AP(nrm.tensor, nrm.offset, [nrm.ap[0], [1, CAST_G], [0, P]]),
                    op0=mybir.AluOpType.mult,
                    op1=mybir.AluOpType.mult,
                )
                nc.sync.dma_start(
                    out=hfull[r0 : r0 + P * CAST_G, :].rearrange(
                        "(p g) d -> p g d", g=CAST_G
                    ),
                    in_=cout[:].rearrange("p (g d) -> p g d", d=P),
                )

            # ---- main loop over dst tiles ----
            for t in range(n_tiles):
                Ct = int(C_t[t])
                icols = int(icols_tb[t].sum())
                mt_i = metap.tile([P, max(icols, 1)], i16, tag="mi")
                ic0 = int(icol_off_tile[t])
                nc.sync.dma_start(out=mt_i[:, :icols], in_=idxm[:, ic0 : ic0 + icols])
                ld_f = metap.tile([P, Cmax], f32, tag="mldf")
                lc0 = int(ld_col_off[t])
                nc.sync.dma_start(out=ld_f[:, :Ct], in_=ldm[:, lc0 : lc0 + Ct])
                ld_bf = metap.tile([P, Cmax], bf16, tag="mldb")
                nc.scalar.copy(out=ld_bf[:, :Ct], in_=ld_f[:, :Ct])

                X_full = xp.tile([P, Cmax * P], bf16, tag="X")
                X = X_full[:, : Ct * P]
                for b in range(NB):
                    n_idx = int(n_tb[t, b])
                    if n_idx == 0:
                        continue
                    co = int(chunk_off_in_tile[t, b])
                    cb = int(ct_tb[t, b])
                    io = int(icol_off_in_tile[t, b])
                    icb = int(icols_tb[t, b])
                    if n_idx % P:
                        # the gather leaves partitions >= n_idx%128 of its
                        # last chunk unwritten; zero that chunk so NaN
                        # garbage can't poison the 0-weighted matmul lanes
                        nc.scalar.memzero(X[:, (co + cb - 1) * P : (co + cb) * P])
                    nc.gpsimd.dma_gather(
                        out_ap=X[:, co * P : (co + cb) * P].rearrange(
                            "p (c e) -> p c e", e=P
                        ),
                        in_ap=hfull[b * B : (b + 1) * B, :],
                        idxs_ap=mt_i[:, io : io + icb],
                        num_idxs=n_idx,
                        num_idxs_reg=n_idx,
                        elem_size=P,
                        single_packet=SINGLE_PACKET,
                        queue_num=b % 4,
                    )

                # wide 0/1 one-hot: S[e, (c,d)] = (iota[d] == ld[e,c])
                S = sp.tile([P, Cmax * P], bf16, tag="S")
                nc.vector.tensor_tensor(
                    out=S[:, : Ct * P].rearrange("p (c d) -> p c d", d=P),
                    in0=AP(
                        iota_bf.tensor, iota_bf.offset, [iota_bf.ap[0], [0, Ct], [1, P]]
                    ),
                    in1=AP(ld_bf.tensor, ld_bf.offset, [ld_bf.ap[0], [1, Ct], [0, P]]),
                    op=mybir.AluOpType.is_equal,
                )

                z_ps = zpsp.tile([P, P], f32)
                for c in range(Ct):
                    # zT[i, d] += X_c[e, i].T @ S_c[e, d]
                    nc.tensor.matmul(
                        out=z_ps[:],
                        lhsT=X[:, c * P : (c + 1) * P],
                        rhs=S[:, c * P : (c + 1) * P],
                        start=(c == 0),
                        stop=(c == Ct - 1),
                    )

                zT_bf = postp.tile([P, P], bf16, tag="zT")
                nc.scalar.copy(out=zT_bf[:], in_=z_ps[:])
                y_ps = ypsp.tile([P, P], f32)
                # y[d, o] = zT[i, d].T @ wt[i, o]
                nc.tensor.matmul(
                    out=y_ps[:], lhsT=zT_bf[:], rhs=wt_bf[:], start=True, stop=True
                )

                rows = P if t < n_tiles - 1 else rows_last
                y_sb = postp.tile([P, P], f32, tag="y")
                # relu(norm_dst * y): post-norm folded into the scale
                nc.scalar.activation(
                    out=y_sb[:],
                    in_=y_ps[:],
                    func=mybir.ActivationFunctionType.Relu,
                    scale=normcol_sb[:, t : t + 1],
                )
                res_sb = postp.tile([P, P], f32, tag="res")
                nc.sync.dma_start(
                    out=res_sb[:rows], in_=resid[t * P : t * P + rows, :]
                )
                o_sb = postp.tile([P, P], f32, tag="o")
                nc.vector.tensor_add(
                    out=o_sb[:rows], in0=y_sb[:rows], in1=res_sb[:rows]
                )
                nc.sync.dma_start(out=out[t * P : t * P + rows, :], in_=o_sb[:rows])
    nc.finalize()
    return nc


def _run(features, W, edge_src, edge_dst, trace=False, **spmd_kwargs):
    in_maps, layout = _prepare(features, W, edge_src, edge_dst)
    nc = _build_program(layout)
    br = run_bass_kernel_spmd(
        nc, in_maps, core_ids=list(range(N_CORES)), trace=trace, **spmd_kwargs
    )
    outs = [r["out"] for r in br.results]
    full = np.concatenate(outs, axis=0).astype(np.float32)
    return full, br


def kernel(features, W, edge_src, edge_dst):
    out, _ = _run(features, W, edge_src, edge_dst, trace=False)
    return out


# revision 5
# speedup vs baseline: 1.9823x; 1.0094x over previous
"""GCN layer (message passing) on 8 Trainium2 NeuronCores.

out = relu(((D^-1/2 A D^-1/2) X) @ W.T) + X

Strategy (dst-sharded graph partitioning, bf16 gather table):
  - Destination nodes sharded across 8 cores (12500 each); every core holds
    the full feature table and computes its 12500 output rows; host concats.
  - Device prologue: cast the f32 feature table to a bf16 DRAM table h with
    the pre-norm D^-1/2 folded in (h[n] = norm[n] * x[n]); partition p casts
    16 consecutive rows per iteration so DMA descriptors stay contiguous.
    The post-norm norm[dst] is folded into the final ReLU's per-partition
    scale, so the one-hot scatter matrices are pure 0/1.
  - Main loop, per dst tile of 128 nodes: up to 4 dma_gather calls (one per
    src bucket of 25088 nodes, int16 indices) pull the edge source rows as
    bf16 into X [128 slots, Ct*128]; idx streams are padded with -1 which
    the gather ucode strips, so each core only moves its actual edge count.
    The one-hot S [128, Ct*128] bf16 is built in ONE wide DVE tensor_tensor
    (iota broadcast along chunks, ld broadcast along the 128 lane dim, both
    via stride-0 APs), then Ct bf16 matmuls accumulate zT[i,d] in PSUM f32,
    y = relu(norm_dst * (zT.T @ W.T)) on ACT, residual add on DVE.
  - Unwritten X slots (cross-core count spread + chunk padding) are
    memzeroed so NaN garbage can't poison the 0-weighted matmul lanes.
"""

import math

import numpy as np

import concourse.bacc as bacc
import concourse.mybir as mybir
from concourse.bass import AP
from concourse.bass_utils import run_bass_kernel_spmd
from concourse.tile import TileContext

P = 128
N_CORES = 8
NB = 4
B = 25088  # bucket size (multiple of 128, int16-indexable)
NPAD = NB * B  # padded node count 100352
CAST_G = 16  # rows per partition per cast iteration
N_NODES = 100000
SINGLE_PACKET = False


def _prepare(features, W, edge_src, edge_dst, n_cores=N_CORES):
    features = np.asarray(features, dtype=np.float32)
    W = np.asarray(W, dtype=np.float32)
    edge_src = np.asarray(edge_src, dtype=np.int32)
    edge_dst = np.asarray(edge_dst, dtype=np.int32)

    n_nodes, d = features.shape
    assert d == P and n_nodes == N_NODES
    npc = n_nodes // n_cores
    n_tiles = math.ceil(npc / P)
    rows_last = npc - (n_tiles - 1) * P

    degs = np.bincount(edge_dst, minlength=n_nodes).astype(np.float32)
    norm = 1.0 / np.sqrt(np.maximum(degs, 1.0), dtype=np.float32)
    norm_pad = np.zeros(NPAD, np.float32)
    norm_pad[:n_nodes] = norm

    featspad = np.zeros((NPAD, P), np.float32)
    featspad[:n_nodes] = features

    # normP[p, j*CAST_G + g] = norm[j*128*CAST_G + p*CAST_G + g]
    n_cast_cols = NPAD // P  # 784
    normP = norm_pad.reshape(n_cast_cols // CAST_G, P, CAST_G)
    normP = np.ascontiguousarray(normP.transpose(1, 0, 2).reshape(P, n_cast_cols))

    core_of = edge_dst // npc

    per_core = []
    counts_all = np.zeros((n_cores, n_tiles, NB), np.int64)
    for k in range(n_cores):
        sel = np.flatnonzero(core_of == k)
        src_k = edge_src[sel]
        ldst = edge_dst[sel] - k * npc
        tile_of = ldst // P
        bucket = src_k // B
        order = np.lexsort((src_k, bucket, tile_of))
        sel = sel[order]
        gid = tile_of[order] * NB + bucket[order]
        counts = np.bincount(gid, minlength=n_tiles * NB).reshape(n_tiles, NB)
        counts_all[k] = counts
        per_core.append((sel, gid, (ldst[order] % P).astype(np.float32)))

    n_tb = counts_all.max(axis=0)  # static gather sizes [n_tiles, NB]
    cnt_min = counts_all.min(axis=0)  # memzero span start
    assert n_tb.sum(axis=1).min() > 0
    ct_tb = (n_tb + P - 1) // P
    C_t = ct_tb.sum(axis=1)
    icols_tb = (n_tb + 15) // 16
    icols_t = icols_tb.sum(axis=1)

    chunk_off_in_tile = np.cumsum(ct_tb, axis=1) - ct_tb
    icol_off_in_tile = np.cumsum(icols_tb, axis=1) - icols_tb
    ld_col_off = np.concatenate([[0], np.cumsum(C_t)])[:-1]
    icol_off_tile = np.concatenate([[0], np.cumsum(icols_t)])[:-1]
    total_icols = int(icols_t.sum())
    total_C = int(C_t.sum())

    layout = dict(
        npc=npc,
        n_tiles=n_tiles,
        rows_last=rows_last,
        n_tb=n_tb,
        cnt_min=cnt_min,
        ct_tb=ct_tb,
        C_t=C_t,
        icols_tb=icols_tb,
        chunk_off_in_tile=chunk_off_in_tile,
        icol_off_in_tile=icol_off_in_tile,
        ld_col_off=ld_col_off,
        icol_off_tile=icol_off_tile,
        total_icols=total_icols,
        total_C=total_C,
        n_cast_cols=n_cast_cols,
    )

    in_maps = []
    wt = np.ascontiguousarray(W.T)
    iotam = np.tile(np.arange(P, dtype=np.float32), (P, 1))
    for k in range(n_cores):
        sel, gid, ld_sorted = per_core[k]
        group_start = np.zeros(n_tiles * NB, np.int64)
        cnts = counts_all[k].reshape(-1)
        group_start[1:] = np.cumsum(cnts)[:-1]
        pos = np.arange(len(sel)) - group_start[gid]
        t_of = gid // NB
        b_of = gid % NB

        # pad with 0 (gathers bucket row 0; killed by ld=-1 in S). -1 padding
        # (ucode strips trailing negatives) hard-crashes the device at scale.
        idx16 = np.zeros((16, total_icols), np.int16)
        icol = icol_off_tile[t_of] + icol_off_in_tile[t_of, b_of] + pos // 16
        idx16[pos % 16, icol] = (edge_src[sel] - b_of * B).astype(np.int16)
        idxm = np.tile(idx16, (8, 1))

        # ld array [128, total_C]: local dst per (chunk col, slot partition)
        ldm = np.full((P, total_C), -1.0, np.float32)
        cit = chunk_off_in_tile[t_of, b_of] + pos // P
        ldm[pos % P, ld_col_off[t_of] + cit] = ld_sorted

        # normcol[p, t] = norm[k*npc + t*128 + p] (own dst rows)
        nslice = np.zeros(n_tiles * P, np.float32)
        nslice[:npc] = norm[k * npc : (k + 1) * npc]
        normcol = np.ascontiguousarray(nslice.reshape(n_tiles, P).T)

        in_maps.append(
            {
                "featspad": featspad,
                "idxm": np.ascontiguousarray(idxm),
                "ldm": np.ascontiguousarray(ldm),
                "wt": wt,
                "iotam": iotam,
                "normP": normP,
                "normcol": normcol,
                "resid": np.ascontiguousarray(features[k * npc : (k + 1) * npc]),
            }
        )
    return in_maps, layout


def _build_program(layout):
    f32 = mybir.dt.float32
    bf16 = mybir.dt.bfloat16
    i16 = mybir.dt.int16
    npc = layout["npc"]
    n_tiles = layout["n_tiles"]
    rows_last = layout["rows_last"]
    n_tb = layout["n_tb"]
    cnt_min = layout["cnt_min"]
    ct_tb = layout["ct_tb"]
    C_t = layout["C_t"]
    icols_tb = layout["icols_tb"]
    chunk_off_in_tile = layout["chunk_off_in_tile"]
    icol_off_in_tile = layout["icol_off_in_tile"]
    ld_col_off = layout["ld_col_off"]
    icol_off_tile = layout["icol_off_tile"]
    n_cast_cols = layout["n_cast_cols"]
    Cmax = int(C_t.max())

    nc = bacc.Bacc(num_swdge_queues=4, dynamic_dma_scratch_size=65536)
    featspad = nc.declare_dram_parameter("featspad", [NPAD, P], f32, isOutput=False)
    idxm = nc.declare_dram_parameter(
        "idxm", [P, layout["total_icols"]], i16, isOutput=False
    )
    ldm = nc.declare_dram_parameter("ldm", [P, layout["total_C"]], f32, isOutput=False)
    wt = nc.declare_dram_parameter("wt", [P, P], f32, isOutput=False)
    iotam = nc.declare_dram_parameter("iotam", [P, P], f32, isOutput=False)
    normP = nc.declare_dram_parameter("normP", [P, n_cast_cols], f32, isOutput=False)
    normcol = nc.declare_dram_parameter("normcol", [P, n_tiles], f32, isOutput=False)
    resid = nc.declare_dram_parameter("resid", [npc, P], f32, isOutput=False)
    out = nc.declare_dram_parameter("out", [npc, P], f32, isOutput=True)

    with TileContext(nc) as tc:
        with (
            tc.tile_pool(name="const", bufs=1) as constp,
            tc.tile_pool(name="hdram", bufs=1, space="DRAM") as hp,
            tc.tile_pool(name="cast", bufs=2) as castp,
            tc.tile_pool(name="meta", bufs=3) as metap,
            tc.tile_pool(name="x", bufs=4) as xp,
            tc.tile_pool(name="s", bufs=3) as sp,
            tc.tile_pool(name="zps", bufs=2, space="PSUM") as zpsp,
            tc.tile_pool(name="yps", bufs=2, space="PSUM") as ypsp,
            tc.tile_pool(name="post", bufs=3) as postp,
        ):
            wt_f = constp.tile([P, P], f32)
            nc.sync.dma_start(out=wt_f[:], in_=wt[:, :])
            wt_bf = constp.tile([P, P], bf16)
            nc.scalar.copy(out=wt_bf[:], in_=wt_f[:])
            iota_f = constp.tile([P, P], f32)
            nc.sync.dma_start(out=iota_f[:], in_=iotam[:, :])
            iota_bf = constp.tile([P, P], bf16)
            nc.scalar.copy(out=iota_bf[:], in_=iota_f[:])
            normP_sb = constp.tile([P, n_cast_cols], f32)
            nc.sync.dma_start(out=normP_sb[:], in_=normP[:, :])
            normcol_sb = constp.tile([P, n_tiles], f32)
            nc.sync.dma_start(out=normcol_sb[:], in_=normcol[:, :])

            hfull = hp.tile([NPAD, P], bf16)

            # ---- prologue: h = norm * x, f32 -> bf16, 2048 rows/iter ----
            for j in range(n_cast_cols // CAST_G):
                r0 = j * P * CAST_G
                cin = castp.tile([P, CAST_G * P], f32, tag="cin")
                nc.sync.dma_start(
                    out=cin[:].rearrange("p (g d) -> p g d", d=P),
                    in_=featspad[r0 : r0 + P * CAST_G, :].rearrange(
                        "(p g) d -> p g d", g=CAST_G
                    ),
                )
                cout = castp.tile([P, CAST_G * P], bf16, tag="cout")
                nrm = normP_sb[:, j * CAST_G : (j + 1) * CAST_G]
                nc.vector.scalar_tensor_tensor(
                    out=cout[:].rearrange("p (g d) -> p g d", d=P),
                    in0=cin[:].rearrange("p (g d) -> p g d", d=P),
                    scalar=1.0,
                    in1=AP(nrm.tensor, nrm.offset, [nrm.ap[0], [1, CAST_G], [0, P]]),
                    op0=mybir.AluOpType.mult,
                    op1=mybir.AluOpType.mult,
                )
                nc.sync.dma_start(
                    out=hfull[r0 : r0 + P * CAST_G, :].rearrange(
                        "(p g) d -> p g d", g=CAST_G
                    ),
                    in_=cout[:].rearrange("p (g d) -> p g d", d=P),
                )

            # ---- main loop over dst tiles ----
            for t in range(n_tiles):
                Ct = int(C_t[t])
                icols = int(icols_tb[t].sum())
                mt_i = metap.tile([P, max(icols, 1)], i16, tag="mi")
                ic0 = int(icol_off_tile[t])
                nc.sync.dma_start(out=mt_i[:, :icols], in_=idxm[:, ic0 : ic0 + icols])
                ld_f = metap.tile([P, Cmax], f32, tag="mldf")
                lc0 = int(ld_col_off[t])
                nc.sync.dma_start(out=ld_f[:, :Ct], in_=ldm[:, lc0 : lc0 + Ct])
                ld_bf = metap.tile([P, Cmax], bf16, tag="mldb")
                nc.scalar.copy(out=ld_bf[:, :Ct], in_=ld_f[:, :Ct])

                X_full = xp.tile([P, Cmax * P], bf16, tag="X")
                X = X_full[:, : Ct * P]
                for b in range(NB):
                    n_idx = int(n_tb[t, b])
                    if n_idx == 0:
                        continue
                    co = int(chunk_off_in_tile[t, b])
                    cb = int(ct_tb[t, b])
                    io = int(icol_off_in_tile[t, b])
                    icb = int(icols_tb[t, b])
                    if n_idx % P:
                        # the gather leaves partitions >= n_idx%128 of its
                        # last chunk unwritten; zero that chunk so NaN
                        # garbage can't poison the 0-weighted matmul lanes
                        nc.scalar.memzero(X[:, (co + cb - 1) * P : (co + cb) * P])
                    nc.gpsimd.dma_gather(
                        out_ap=X[:, co * P : (co + cb) * P].rearrange(
                            "p (c e) -> p c e", e=P
                        ),
                        in_ap=hfull[b * B : (b + 1) * B, :],
                        idxs_ap=mt_i[:, io : io + icb],
                        num_idxs=n_idx,
                        num_idxs_reg=n_idx,
                        elem_size=P,
                        single_packet=SINGLE_PACKET,
                        queue_num=b % 4,
                    )

                # wide 0/1 one-hot: S[e, (c,d)] = (iota[d] == ld[e,c])
                S = sp.tile([P, Cmax * P], bf16, tag="S")
                nc.vector.tensor_tensor(
                    out=S[:, : Ct * P].rearrange("p (c d) -> p c d", d=P),
                    in0=AP(
                        iota_bf.tensor, iota_bf.offset, [iota_bf.ap[0], [0, Ct], [1, P]]
                    ),
                    in1=AP(ld_bf.tensor, ld_bf.offset, [ld_bf.ap[0], [1, Ct], [0, P]]),
                    op=mybir.AluOpType.is_equal,
                )

                z_ps = zpsp.tile([P, P], f32)
                for c in range(Ct):
                    # zT[i, d] += X_c[e, i].T @ S_c[e, d]
                    nc.tensor.matmul(
                        out=z_ps[:],
                        lhsT=X[:, c * P : (c + 1) * P],
                        rhs=S[:, c * P : (c + 1) * P],
                        start=(c == 0),
                        stop=(c == Ct - 1),
                    )

                zT_bf = postp.tile([P, P], bf16, tag="zT")
                nc.scalar.copy(out=zT_bf[:], in_=z_ps[:])
                y_ps = ypsp.tile([P, P], f32)
                # y[d, o] = zT[i, d].T @ wt[i, o]
                nc.tensor.matmul(
                    out=y_ps[:], lhsT=zT_bf[:], rhs=wt_bf[:], start=True, stop=True
                )

                rows = P if t < n_tiles - 1 else rows_last
                y_sb = postp.tile([P, P], f32, tag="y")
                # relu(norm_dst * y): post-norm folded into the scale
                nc.scalar.activation(
                    out=y_sb[:],
                    in_=y_ps[:],
                    func=mybir.ActivationFunctionType.Relu,
                    scale=normcol_sb[:, t : t + 1],
                )
                res_sb = postp.tile([P, P], f32, tag="res")
                nc.sync.dma_start(
                    out=res_sb[:rows], in_=resid[t * P : t * P + rows, :]
                )
                o_sb = postp.tile([P, P], f32, tag="o")
                nc.vector.tensor_add(
                    out=o_sb[:rows], in0=y_sb[:rows], in1=res_sb[:rows]
                )
                nc.sync.dma_start(out=out[t * P : t * P + rows, :], in_=o_sb[:rows])
    nc.finalize()
    return nc


def _run(features, W, edge_src, edge_dst, trace=False, **spmd_kwargs):
    in_maps, layout = _prepare(features, W, edge_src, edge_dst)
    nc = _build_program(layout)
    br = run_bass_kernel_spmd(
        nc, in_maps, core_ids=list(range(N_CORES)), trace=trace, **spmd_kwargs
    )
    outs = [r["out"] for r in br.results]
    full = np.concatenate(outs, axis=0).astype(np.float32)
    return full, br


def kernel(features, W, edge_src, edge_dst):
    out, _ = _run(features, W, edge_src, edge_dst, trace=False)
    return out
